# revision 2
# baseline (speedup 1.0000x reference)
"""Bass/Tile TRN2 kernel for nn_BatchAdditiveAttention.

Math (per batch, per node n):
    f_proj      = feature @ w1.T                        # (n, 128)
    t_proj[r]   = temb[:, r] @ w2.T                     # (n, 4, 128)
    q[r]        = tanh(f_proj + t_proj[r])              # (n, 4, 128)
    score[r]    = q[r] @ m                              # (n, 4)
    beta        = softmax_r(score)                      # (n, 4)
    out         = sum_r beta[r] * temb[:, r]            # (n, 256)

Sharding: data-parallel over bs=8, one batch per NeuronCore.

Layout strategy (v2): the host pre-casts the two big inputs to bf16 and
also pre-transposes copies of them (d on the leading axis), so the
device reads:
  - temb_t / feat_t  [.., 128, N] bf16  -> moving operands for the
    projection matmuls (d on partitions), no on-chip transposes at all
  - temb_n           [N, R, D] bf16     -> natural-layout operand for
    the beta-weighted output reduction (diag(exp) stationary trick)
The output is stored bf16 and upcast on the host.  This removes all PE
transposes + PSUM->SBUF copybacks of v1 and cuts device HBM traffic to
~102 MB/core (temb read twice: once transposed for the projections,
once natural for the output reduction; that is still far cheaper than
transposing 51 MB/core on-chip).

Softmax skips the max-subtraction: |score| <= ||m||_1 <= 11.4, so exp
stays comfortably inside f32/bf16 range and matches the reference well
within the 2e-2 gate.
"""

import os
from contextlib import ExitStack

import numpy as np
import ml_dtypes

import concourse.bass as bass
import concourse.tile as tile
from concourse import bacc, mybir

BS = 8
N_NODES = 20000
D = 256
R = 4
D2 = 128
NT = 512  # nodes per tile
PB = 128  # nodes per sub-tile (partition block)

BF16 = mybir.dt.bfloat16
F32 = mybir.dt.float32
AX = mybir.AxisListType
ALU = mybir.AluOpType
ACTF = mybir.ActivationFunctionType


def _sub_blocks(nt):
    """Split a node-tile of nt nodes into partition blocks of <=128."""
    blocks = []
    off = 0
    while off < nt:
        blocks.append((off // PB, min(PB, nt - off)))
        off += PB
    return blocks


DEFAULT_OPTS = dict(
    io_bufs=4,
    q_bufs=6,
    qp_bufs=3,
    fp_bufs=3,
    sc_bufs=2,
    o_bufs=3,
    negmax=False,    # subtract running max before exp (off: scores bounded)
    act_osb=False,   # do the final 1/sum scaling on ACT instead of DVE
    gp_loads=False,  # issue the loads on SWDGE (gpsimd) instead of HWDGE
)


def build_kernel_body_pt(ctx, tc, n_nodes, aps, opts=None, time_reps=None):
    o = dict(DEFAULT_OPTS, **(opts or {}))
    nc = tc.nc
    temb_n, temb_t, feat_t, w1t, w2t, mcol, eye, out = aps

    const = ctx.enter_context(tc.tile_pool(name="const", bufs=1))
    tio = ctx.enter_context(tc.tile_pool(name="tio", bufs=o["io_bufs"]))
    ttio = ctx.enter_context(tc.tile_pool(name="ttio", bufs=o["io_bufs"]))
    ftio = ctx.enter_context(tc.tile_pool(name="ftio", bufs=o["io_bufs"]))
    qpool = ctx.enter_context(tc.tile_pool(name="qpool", bufs=o["q_bufs"]))
    small = ctx.enter_context(tc.tile_pool(name="small", bufs=4))
    opool = ctx.enter_context(tc.tile_pool(name="opool", bufs=o["o_bufs"]))
    qpsum = ctx.enter_context(tc.tile_pool(name="qpsum", bufs=o["qp_bufs"], space="PSUM"))
    spsum = ctx.enter_context(tc.tile_pool(name="spsum", bufs=o["sc_bufs"], space="PSUM"))
    fpsum = ctx.enter_context(tc.tile_pool(name="fpsum", bufs=o["fp_bufs"], space="PSUM"))

    w1sb = const.tile([128, 2, D2], BF16)
    w2sb = const.tile([128, 2, D2], BF16)
    msb = const.tile([128, 1], BF16)
    eyesb = const.tile([128, 128], BF16)
    for c in range(2):
        nc.sync.dma_start(out=w1sb[:, c, :], in_=w1t[c])
        nc.sync.dma_start(out=w2sb[:, c, :], in_=w2t[c])
    nc.sync.dma_start(out=msb[:], in_=mcol[:])
    nc.sync.dma_start(out=eyesb[:], in_=eye[:])

    load_eng = nc.gpsimd if o["gp_loads"] else nc.sync

    rep_cm = tc.For_i(0, time_reps, 1) if time_reps else None
    if rep_cm is not None:
        ctx.enter_context(rep_cm)
    for t0 in range(0, n_nodes, NT):
        nt = min(NT, n_nodes - t0)
        blocks = _sub_blocks(nt)
        na = len(blocks)
        p = min(PB, nt)

        tn = tio.tile([128, 4, R, D], BF16, tag="tn")
        load_eng.dma_start(
            out=tn[0:p, 0:na, :, :],
            in_=temb_n[t0 : t0 + nt].rearrange("(a p) r d -> p a r d", p=p),
        )
        tt = ttio.tile([128, R, 2, NT], BF16, tag="tt")
        load_eng.dma_start(
            out=tt[:, :, :, 0:nt],
            in_=temb_t[:, :, :, t0 : t0 + nt].rearrange("r c p n -> p r c n"),
        )
        ft = ftio.tile([128, 2, NT], BF16, tag="ft")
        load_eng.dma_start(
            out=ft[:, :, 0:nt],
            in_=feat_t[:, :, t0 : t0 + nt].rearrange("c p n -> p c n"),
        )

        scores = spsum.tile([128, 4 * R], F32, tag="sc")
        for r in range(R):
            qp = qpsum.tile([128, NT], F32, tag="qp")
            nc.tensor.matmul(qp[:, 0:nt], w1sb[:, 0, :], ft[:, 0, 0:nt],
                             start=True, stop=False)
            nc.tensor.matmul(qp[:, 0:nt], w1sb[:, 1, :], ft[:, 1, 0:nt],
                             start=False, stop=False)
            nc.tensor.matmul(qp[:, 0:nt], w2sb[:, 0, :], tt[:, r, 0, 0:nt],
                             start=False, stop=False)
            nc.tensor.matmul(qp[:, 0:nt], w2sb[:, 1, :], tt[:, r, 1, 0:nt],
                             start=False, stop=True)

            q = qpool.tile([128, NT], BF16, tag="q")
            nc.scalar.activation(q[:, 0:nt], qp[:, 0:nt], ACTF.Tanh)

            for a, ns in blocks:
                nc.tensor.matmul(
                    scores[0:ns, a * R + r : a * R + r + 1],
                    q[:, a * PB : a * PB + ns],
                    msb[:, 0:1],
                    start=True, stop=True,
                )

        osb = opool.tile([128, 4, D], BF16, tag="osb")
        for a, ns in blocks:
            sc = scores[0:ns, a * R : (a + 1) * R]
            expo = small.tile([128, R], F32, tag="expo")
            sume = small.tile([128, 1], F32, tag="sume")
            if o["negmax"]:
                negmax = small.tile([128, 1], F32, tag="negmax")
                nc.vector.tensor_reduce(negmax[0:ns], sc, AX.X, ALU.max,
                                        negate=True)
                nc.scalar.activation(expo[0:ns], sc, ACTF.Exp,
                                     bias=negmax[0:ns], accum_out=sume[0:ns])
            else:
                nc.scalar.activation(expo[0:ns], sc, ACTF.Exp,
                                     accum_out=sume[0:ns])
            inv = small.tile([128, 1], F32, tag="inv")
            nc.vector.reciprocal(inv[0:ns], sume[0:ns])

            fp = fpsum.tile([128, D], F32, tag="fp")
            for r in range(R):
                diag = small.tile([128, 128], BF16, tag="diag")
                nc.vector.tensor_scalar_mul(
                    diag[0:ns, 0:ns], eyesb[0:ns, 0:ns], expo[0:ns, r : r + 1]
                )
                nc.tensor.matmul(fp[0:ns, :], diag[0:ns, 0:ns],
                                 tn[0:ns, a, r, :],
                                 start=(r == 0), stop=(r == R - 1))
            if o["act_osb"]:
                nc.scalar.activation(osb[0:ns, a, :], fp[0:ns, :], ACTF.Copy,
                                     scale=inv[0:ns, 0:1])
            else:
                nc.vector.tensor_scalar_mul(osb[0:ns, a, :], fp[0:ns, :],
                                            inv[0:ns, 0:1])

        nc.sync.dma_start(
            out=out[t0 : t0 + nt].rearrange("(a p) d -> p a d", p=p),
            in_=osb[0:p, 0:na, :],
        )


def build_program_pt(n_nodes=N_NODES, num_devices=BS, opts=None, time_reps=None):
    nc = bacc.Bacc(
        "TRN2", target_bir_lowering=False, debug=False, num_devices=num_devices
    )
    temb_n = nc.dram_tensor("temb_n", [n_nodes, R, D], BF16, kind="ExternalInput").ap()
    temb_t = nc.dram_tensor(
        "temb_t", [R, 2, 128, n_nodes], BF16, kind="ExternalInput"
    ).ap()
    feat_t = nc.dram_tensor("feat_t", [2, 128, n_nodes], BF16, kind="ExternalInput").ap()
    w1t = nc.dram_tensor("w1t", [2, 128, D2], BF16, kind="ExternalInput").ap()
    w2t = nc.dram_tensor("w2t", [2, 128, D2], BF16, kind="ExternalInput").ap()
    mcol = nc.dram_tensor("mcol", [D2, 1], BF16, kind="ExternalInput").ap()
    eye = nc.dram_tensor("eye", [128, 128], BF16, kind="ExternalInput").ap()
    out = nc.dram_tensor("out", [n_nodes, D], BF16, kind="ExternalOutput").ap()

    with tile.TileContext(nc) as tc, ExitStack() as ctx:
        build_kernel_body_pt(
            ctx, tc, n_nodes,
            (temb_n, temb_t, feat_t, w1t, w2t, mcol, eye, out), opts=opts,
            time_reps=time_reps,
        )
    nc.compile()
    return nc


def make_pt_inputs(feature, temb):
    """Host-side bf16 cast + transpose.
    feature: (bs, N, D) f32; temb: (bs, N, R, D) f32."""
    bf = ml_dtypes.bfloat16
    bs, n = feature.shape[0], feature.shape[1]
    temb_bf = temb.astype(bf)
    temb_n = temb_bf  # (bs, N, R, D)
    temb_t = np.ascontiguousarray(
        temb_bf.reshape(bs, n, R, 2, 128).transpose(0, 2, 3, 4, 1)
    )  # (bs, R, 2, 128, N)
    feat_t = np.ascontiguousarray(
        feature.astype(bf).reshape(bs, n, 2, 128).transpose(0, 2, 3, 1)
    )  # (bs, 2, 128, N)
    return temb_n, temb_t, feat_t


def make_const_inputs(w1, w2, m):
    bf = ml_dtypes.bfloat16
    w1t = np.ascontiguousarray(w1.T.astype(bf)).reshape(2, 128, D2)
    w2t = np.ascontiguousarray(w2.T.astype(bf)).reshape(2, 128, D2)
    mcol = np.ascontiguousarray(m.reshape(D2, 1).astype(bf))
    eye = np.eye(128, dtype=bf)
    return w1t, w2t, mcol, eye


_cached_nc = None
_cached_opts = None


def kernel(feature, type_aware_emb, w1, w2, m, _trace=False, _tmpdir=None,
           _opts=None):
    from concourse.bass_utils import run_bass_kernel_spmd

    global _cached_nc, _cached_opts
    if _cached_nc is None or _opts != _cached_opts:
        _cached_nc = build_program_pt(opts=_opts)
        _cached_opts = _opts
    nc = _cached_nc

    w1t, w2t, mcol, eye = make_const_inputs(
        np.asarray(w1, np.float32), np.asarray(w2, np.float32),
        np.asarray(m, np.float32),
    )
    feature = np.asarray(feature, np.float32)
    temb = np.asarray(type_aware_emb, np.float32)
    temb_n, temb_t, feat_t = make_pt_inputs(feature, temb)
    in_maps = [
        {
            "temb_n": temb_n[i],
            "temb_t": temb_t[i],
            "feat_t": feat_t[i],
            "w1t": w1t,
            "w2t": w2t,
            "mcol": mcol,
            "eye": eye,
        }
        for i in range(BS)
    ]
    res = run_bass_kernel_spmd(
        nc, in_maps, list(range(BS)), trace=_trace, tmpdir=_tmpdir
    )
    out = np.stack([np.asarray(res.results[i]["out"]) for i in range(BS)])
    if _trace:
        kernel.last_result = res
    return out.reshape(BS, N_NODES, 1, D).astype(np.float32)


# revision 5
# speedup vs baseline: 1.2076x; 1.2076x over previous
"""Bass/Tile TRN2 kernel for nn_BatchAdditiveAttention.

Math (per batch, per node n):
    f_proj      = feature @ w1.T                        # (n, 128)
    t_proj[r]   = temb[:, r] @ w2.T                     # (n, 4, 128)
    q[r]        = tanh(f_proj + t_proj[r])              # (n, 4, 128)
    score[r]    = q[r] @ m                              # (n, 4)
    beta        = softmax_r(score)                      # (n, 4)
    out         = sum_r beta[r] * temb[:, r]            # (n, 256)

Sharding: data-parallel over bs=8, one batch per NeuronCore.

Layout strategy (v2): the host pre-casts the two big inputs to bf16 and
also pre-transposes copies of them (d on the leading axis), so the
device reads:
  - temb_t / feat_t  [.., 128, N] bf16  -> moving operands for the
    projection matmuls (d on partitions), no on-chip transposes at all
  - temb_n           [N, R, D] bf16     -> natural-layout operand for
    the beta-weighted output reduction (diag(exp) stationary trick)
The output is stored bf16 and upcast on the host.  This removes all PE
transposes + PSUM->SBUF copybacks of v1 and cuts device HBM traffic to
~102 MB/core (temb read twice: once transposed for the projections,
once natural for the output reduction; that is still far cheaper than
transposing 51 MB/core on-chip).

Softmax skips the max-subtraction: |score| <= ||m||_1 <= 11.4, so exp
stays comfortably inside f32/bf16 range and matches the reference well
within the 2e-2 gate.
"""

import os
from contextlib import ExitStack

import numpy as np
import ml_dtypes

import concourse.bass as bass
import concourse.tile as tile
from concourse import bacc, mybir

BS = 8
N_NODES = 20000
D = 256
R = 4
D2 = 128
NT = 512  # nodes per tile
PB = 128  # nodes per sub-tile (partition block)

BF16 = mybir.dt.bfloat16
F32 = mybir.dt.float32
AX = mybir.AxisListType
ALU = mybir.AluOpType
ACTF = mybir.ActivationFunctionType


def _sub_blocks(nt):
    """Split a node-tile of nt nodes into partition blocks of <=128."""
    blocks = []
    off = 0
    while off < nt:
        blocks.append((off // PB, min(PB, nt - off)))
        off += PB
    return blocks


DEFAULT_OPTS = dict(
    io_bufs=3,       # buffers of PAIR-sized (2*NT) load tiles
    q_bufs=6,
    qp_bufs=4,
    fp_bufs=2,
    sc_bufs=2,
    o_bufs=3,
    negmax=False,    # subtract running max before exp (off: scores bounded)
    act_osb=False,   # do the final 1/sum scaling on ACT instead of DVE
    gp_loads=True,   # issue the loads on SWDGE (gpsimd) instead of HWDGE
    wmajor=True,     # weight-major qproj: 4 LDWEIGHTS/tile instead of 16
    pipe=True,       # emit tile k's softmax+out after tile k+1's projection
)


def build_kernel_body_pt(ctx, tc, n_nodes, aps, opts=None, time_reps=None):
    o = dict(DEFAULT_OPTS, **(opts or {}))
    nc = tc.nc
    temb_n, temb_t, feat_t, w1t, w2t, mcol, eye, out = aps

    const = ctx.enter_context(tc.tile_pool(name="const", bufs=1))
    tio = ctx.enter_context(tc.tile_pool(name="tio", bufs=o["io_bufs"]))
    ttio = ctx.enter_context(tc.tile_pool(name="ttio", bufs=o["io_bufs"]))
    ftio = ctx.enter_context(tc.tile_pool(name="ftio", bufs=o["io_bufs"]))
    qpool = ctx.enter_context(tc.tile_pool(name="qpool", bufs=o["q_bufs"]))
    small = ctx.enter_context(tc.tile_pool(name="small", bufs=4))
    opool = ctx.enter_context(tc.tile_pool(name="opool", bufs=o["o_bufs"]))
    qpsum = ctx.enter_context(tc.tile_pool(name="qpsum", bufs=o["qp_bufs"], space="PSUM"))
    spsum = ctx.enter_context(tc.tile_pool(name="spsum", bufs=o["sc_bufs"], space="PSUM"))
    fpsum = ctx.enter_context(tc.tile_pool(name="fpsum", bufs=o["fp_bufs"], space="PSUM"))

    w1sb = const.tile([128, 2, D2], BF16)
    w2sb = const.tile([128, 2, D2], BF16)
    msb = const.tile([128, 1], BF16)
    eyesb = const.tile([128, 128], BF16)
    for c in range(2):
        nc.sync.dma_start(out=w1sb[:, c, :], in_=w1t[c])
        nc.sync.dma_start(out=w2sb[:, c, :], in_=w2t[c])
    nc.sync.dma_start(out=msb[:], in_=mcol[:])
    nc.sync.dma_start(out=eyesb[:], in_=eye[:])

    load_eng = nc.gpsimd if o["gp_loads"] else nc.sync
    PAIR = 2 * NT

    rep_cm = tc.For_i(0, time_reps, 1) if time_reps else None
    if rep_cm is not None:
        ctx.enter_context(rep_cm)

    def stage_A(tn2, tt2, ft2, h, t0, nt):
        """Projection + scores for one NT-tile (half h of its pair)."""
        blocks = _sub_blocks(nt)
        n0 = h * NT
        scores = spsum.tile([128, 4 * R], F32, tag="sc")
        qps = []
        if o["wmajor"]:
            for r in range(R):
                qps.append(qpsum.tile([128, NT], F32, tag="qp", name="qp"))
            wmms = (
                [(w2sb[:, c, :], lambda r, c=c: tt2[:, r, c, n0 : n0 + nt])
                 for c in range(2)]
                + [(w1sb[:, c, :], lambda r, c=c: ft2[:, c, n0 : n0 + nt])
                   for c in range(2)]
            )
            for wi, (wsb, mov) in enumerate(wmms):
                for r in range(R):
                    nc.tensor.matmul(qps[r][:, 0:nt], wsb, mov(r),
                                     start=(wi == 0), stop=(wi == 3))
        else:
            for r in range(R):
                qp = qpsum.tile([128, NT], F32, tag="qp")
                nc.tensor.matmul(qp[:, 0:nt], w1sb[:, 0, :],
                                 ft2[:, 0, n0 : n0 + nt], start=True, stop=False)
                nc.tensor.matmul(qp[:, 0:nt], w1sb[:, 1, :],
                                 ft2[:, 1, n0 : n0 + nt], start=False, stop=False)
                nc.tensor.matmul(qp[:, 0:nt], w2sb[:, 0, :],
                                 tt2[:, r, 0, n0 : n0 + nt], start=False, stop=False)
                nc.tensor.matmul(qp[:, 0:nt], w2sb[:, 1, :],
                                 tt2[:, r, 1, n0 : n0 + nt], start=False, stop=True)
                qps.append(qp)
        for r in range(R):
            q = qpool.tile([128, NT], BF16, tag="q")
            nc.scalar.activation(q[:, 0:nt], qps[r][:, 0:nt], ACTF.Tanh)
            for a, ns in blocks:
                nc.tensor.matmul(
                    scores[0:ns, a * R + r : a * R + r + 1],
                    q[:, a * PB : a * PB + ns],
                    msb[:, 0:1],
                    start=True, stop=True,
                )
        return scores

    def stage_B(tn2, scores, h, t0, nt):
        """Softmax + beta-weighted output + store for one NT-tile."""
        blocks = _sub_blocks(nt)
        na = len(blocks)
        p = min(PB, nt)
        osb = opool.tile([128, 4, D], BF16, tag="osb")
        for a, ns in blocks:
            sc = scores[0:ns, a * R : (a + 1) * R]
            expo = small.tile([128, R], F32, tag="expo")
            sume = small.tile([128, 1], F32, tag="sume")
            if o["negmax"]:
                negmax = small.tile([128, 1], F32, tag="negmax")
                nc.vector.tensor_reduce(negmax[0:ns], sc, AX.X, ALU.max,
                                        negate=True)
                nc.scalar.activation(expo[0:ns], sc, ACTF.Exp,
                                     bias=negmax[0:ns], accum_out=sume[0:ns])
            else:
                nc.scalar.activation(expo[0:ns], sc, ACTF.Exp,
                                     accum_out=sume[0:ns])
            inv = small.tile([128, 1], F32, tag="inv")
            nc.vector.reciprocal(inv[0:ns], sume[0:ns])

            fp = fpsum.tile([128, D], F32, tag="fp")
            for r in range(R):
                diag = small.tile([128, 128], BF16, tag="diag")
                nc.vector.tensor_scalar_mul(
                    diag[0:ns, 0:ns], eyesb[0:ns, 0:ns], expo[0:ns, r : r + 1]
                )
                nc.tensor.matmul(fp[0:ns, :], diag[0:ns, 0:ns],
                                 tn2[0:ns, 4 * h + a, r, :],
                                 start=(r == 0), stop=(r == R - 1))
            if o["act_osb"]:
                nc.scalar.activation(osb[0:ns, a, :], fp[0:ns, :], ACTF.Copy,
                                     scale=inv[0:ns, 0:1])
            else:
                nc.vector.tensor_scalar_mul(osb[0:ns, a, :], fp[0:ns, :],
                                            inv[0:ns, 0:1])
        nc.sync.dma_start(
            out=out[t0 : t0 + nt].rearrange("(a p) d -> p a d", p=p),
            in_=osb[0:p, 0:na, :],
        )

    pending = None
    for p0 in range(0, n_nodes, PAIR):
        bnt = min(PAIR, n_nodes - p0)
        bp = min(PB, bnt)
        bna = (bnt + PB - 1) // PB
        tn2 = tio.tile([128, 8, R, D], BF16, tag="tn")
        tt2 = ttio.tile([128, R, 2, PAIR], BF16, tag="tt")
        ft2 = ftio.tile([128, 2, PAIR], BF16, tag="ft")
        if bnt == PAIR:
            load_eng.dma_start(
                out=tn2[0:bp, 0:bna, :, :],
                in_=temb_n[p0 : p0 + bnt].rearrange("(a p) r d -> p a r d", p=bp),
            )
            load_eng.dma_start(
                out=tt2[:, :, :, 0:bnt],
                in_=temb_t[:, :, :, p0 : p0 + bnt].rearrange("r c p n -> p r c n"),
            )
            load_eng.dma_start(
                out=ft2[:, :, 0:bnt],
                in_=feat_t[:, :, p0 : p0 + bnt].rearrange("c p n -> p c n"),
            )
        else:
            for h in range(2):
                t0 = p0 + h * NT
                if t0 >= n_nodes:
                    break
                nt = min(NT, n_nodes - t0)
                hp = min(PB, nt)
                hna = (nt + PB - 1) // PB
                load_eng.dma_start(
                    out=tn2[0:hp, 4 * h : 4 * h + hna, :, :],
                    in_=temb_n[t0 : t0 + nt].rearrange("(a p) r d -> p a r d", p=hp),
                )
                load_eng.dma_start(
                    out=tt2[:, :, :, h * NT : h * NT + nt],
                    in_=temb_t[:, :, :, t0 : t0 + nt].rearrange("r c p n -> p r c n"),
                )
                load_eng.dma_start(
                    out=ft2[:, :, h * NT : h * NT + nt],
                    in_=feat_t[:, :, t0 : t0 + nt].rearrange("c p n -> p c n"),
                )
        for h in range(2):
            t0 = p0 + h * NT
            if t0 >= n_nodes:
                break
            nt = min(NT, n_nodes - t0)
            scores = stage_A(tn2, tt2, ft2, h, t0, nt)
            if not o["pipe"]:
                stage_B(tn2, scores, h, t0, nt)
            else:
                if pending is not None:
                    stage_B(*pending)
                pending = (tn2, scores, h, t0, nt)
    if pending is not None:
        stage_B(*pending)


def build_program_pt(n_nodes=N_NODES, num_devices=BS, opts=None, time_reps=None):
    nc = bacc.Bacc(
        "TRN2", target_bir_lowering=False, debug=False, num_devices=num_devices
    )
    temb_n = nc.dram_tensor("temb_n", [n_nodes, R, D], BF16, kind="ExternalInput").ap()
    temb_t = nc.dram_tensor(
        "temb_t", [R, 2, 128, n_nodes], BF16, kind="ExternalInput"
    ).ap()
    feat_t = nc.dram_tensor("feat_t", [2, 128, n_nodes], BF16, kind="ExternalInput").ap()
    w1t = nc.dram_tensor("w1t", [2, 128, D2], BF16, kind="ExternalInput").ap()
    w2t = nc.dram_tensor("w2t", [2, 128, D2], BF16, kind="ExternalInput").ap()
    mcol = nc.dram_tensor("mcol", [D2, 1], BF16, kind="ExternalInput").ap()
    eye = nc.dram_tensor("eye", [128, 128], BF16, kind="ExternalInput").ap()
    out = nc.dram_tensor("out", [n_nodes, D], BF16, kind="ExternalOutput").ap()

    with tile.TileContext(nc) as tc, ExitStack() as ctx:
        build_kernel_body_pt(
            ctx, tc, n_nodes,
            (temb_n, temb_t, feat_t, w1t, w2t, mcol, eye, out), opts=opts,
            time_reps=time_reps,
        )
    nc.compile()
    return nc


def make_pt_inputs(feature, temb):
    """Host-side bf16 cast + transpose.
    feature: (bs, N, D) f32; temb: (bs, N, R, D) f32."""
    bf = ml_dtypes.bfloat16
    bs, n = feature.shape[0], feature.shape[1]
    temb_bf = temb.astype(bf)
    temb_n = temb_bf  # (bs, N, R, D)
    temb_t = np.ascontiguousarray(
        temb_bf.reshape(bs, n, R, 2, 128).transpose(0, 2, 3, 4, 1)
    )  # (bs, R, 2, 128, N)
    feat_t = np.ascontiguousarray(
        feature.astype(bf).reshape(bs, n, 2, 128).transpose(0, 2, 3, 1)
    )  # (bs, 2, 128, N)
    return temb_n, temb_t, feat_t


def make_const_inputs(w1, w2, m):
    bf = ml_dtypes.bfloat16
    w1t = np.ascontiguousarray(w1.T.astype(bf)).reshape(2, 128, D2)
    w2t = np.ascontiguousarray(w2.T.astype(bf)).reshape(2, 128, D2)
    mcol = np.ascontiguousarray(m.reshape(D2, 1).astype(bf))
    eye = np.eye(128, dtype=bf)
    return w1t, w2t, mcol, eye


_cached_nc = None
_cached_opts = None


def kernel(feature, type_aware_emb, w1, w2, m, _trace=False, _tmpdir=None,
           _opts=None):
    from concourse.bass_utils import run_bass_kernel_spmd

    global _cached_nc, _cached_opts
    if _cached_nc is None or _opts != _cached_opts:
        _cached_nc = build_program_pt(opts=_opts)
        _cached_opts = _opts
    nc = _cached_nc

    w1t, w2t, mcol, eye = make_const_inputs(
        np.asarray(w1, np.float32), np.asarray(w2, np.float32),
        np.asarray(m, np.float32),
    )
    feature = np.asarray(feature, np.float32)
    temb = np.asarray(type_aware_emb, np.float32)
    temb_n, temb_t, feat_t = make_pt_inputs(feature, temb)
    in_maps = [
        {
            "temb_n": temb_n[i],
            "temb_t": temb_t[i],
            "feat_t": feat_t[i],
            "w1t": w1t,
            "w2t": w2t,
            "mcol": mcol,
            "eye": eye,
        }
        for i in range(BS)
    ]
    res = run_bass_kernel_spmd(
        nc, in_maps, list(range(BS)), trace=_trace, tmpdir=_tmpdir
    )
    out = np.stack([np.asarray(res.results[i]["out"]) for i in range(BS)])
    if _trace:
        kernel.last_result = res
    return out.reshape(BS, N_NODES, 1, D).astype(np.float32)
